# revision 1
# baseline (speedup 1.0000x reference)
"""MultiHeadAttention Trainium2 Bass kernel, 8-core SPMD.

Problem: B=4, S=2048, E=2048, H=16, Dh=128; reshape-based (not transposed)
head split:  q = (x@Wq).reshape(B,H,S,Dh) etc., softmax over the QUERY axis,
out = attn.reshape(B,S,E).

Key structure: flattening (B,S) rows, row-block gp (128 rows) of x@W is
exactly head pair gp=(b,h): Qh = Y[128gp:128gp+128,:].reshape(2048,128).
Each of the 8 cores handles 8 consecutive pairs -> core c gets contiguous
x rows [1024c:1024c+1024) and produces the same output rows. No collectives.

Per-core internal q/k index permutation (order-free since softmax reduces
over q): f = j*128 + s  <->  q = 16s + j. With that permutation:
  QT/KT [d, f]  = the j-th 128-col block of (Xblk @ W)^T, stored contiguous
  Vh block kj   = rows of Yv = Xblk@Wv in natural [s, e] layout, e-block kj
  out block     = per-128-col transpose of attnT.

Dtypes: fp32r (TF32-like, 1cy/row at N>=256) for projections + scores;
bf16 for softmax weights + attn; fp32 PSUM/softmax stats throughout.
Measured end-to-end numerics (numpy sim): rel L2 ~ 3.5e-3 vs fp32 ref.
"""

import numpy as np
from contextlib import ExitStack

import concourse.bass as bass
import concourse.tile as tile
from concourse import bacc, mybir
from concourse.bass import ds, ts
from concourse.bass_utils import run_bass_kernel_spmd
from concourse.masks import make_identity

F32 = mybir.dt.float32
F32R = mybir.dt.float32r
BF16 = mybir.dt.bfloat16
AX = mybir.AxisListType.X
EXP = mybir.ActivationFunctionType.Exp

P = 128
NPAIR = 8          # (b,h) pairs per core
GRP = 4            # pairs per phase group
NGRP = NPAIR // GRP
NJ = 16            # 128-blocks in E / contraction
G = 6              # max kj per attn accumulation group (groups 6,6,4)
GROUP_START = {5: 0, 11: 6, 15: 12}   # kj at group end -> group start
SCALE = 1.0 / np.sqrt(128.0)

_cache = {}


def _emit(nc, tc, ctx, xl, wq, wk, wv, idr, out, reps=1, parts="abc"):
    sb = ctx.enter_context
    pIN = sb(tc.tile_pool(name="pin", bufs=1))
    pXT = sb(tc.tile_pool(name="pxt", bufs=1))
    pW = sb(tc.tile_pool(name="pw", bufs=2))
    pSTG = sb(tc.tile_pool(name="pstg", bufs=2))
    pYV = sb(tc.tile_pool(name="pyv", bufs=4))
    pQK = sb(tc.tile_pool(name="pqk", bufs=2))
    pSOFT = sb(tc.tile_pool(name="psoft", bufs=G + 1))
    pVS = sb(tc.tile_pool(name="pvs", bufs=G + 1))
    pACC = sb(tc.tile_pool(name="pacc", bufs=2))
    pST = sb(tc.tile_pool(name="pst", bufs=8))
    pCONST = sb(tc.tile_pool(name="pconst", bufs=1))
    psSC = sb(tc.tile_pool(name="pssc", bufs=2, space="PSUM"))   # [128,1024] x2 = 4 banks
    psAT = sb(tc.tile_pool(name="psat", bufs=1, space="PSUM"))   # [128,1024]    = 2 banks
    psMX = sb(tc.tile_pool(name="psmx", bufs=2, space="PSUM"))   # [128,512] x2  = 2 banks
    dram = sb(tc.tile_pool(name="dram", bufs=1, space="DRAM"))

    qsp = dram.tile([P, NPAIR, NJ, P], F32R, tag="qsp")
    ksp = dram.tile([P, NPAIR, NJ, P], F32R, tag="ksp")

    ident = pCONST.tile([P, P], F32, tag="ident")
    make_identity(nc, ident[:])
    identr = pCONST.tile([P, P], F32R, tag="identr")
    nc.sync.dma_start(identr[:], idr)
    ident_r = identr[:]

    yv_tiles = {}

    def phase_a(grp):
        """Transpose the group's x blocks into XTg [P, kb, pair, s] (f32r)."""
        xtg = pXT.tile([P, NJ, GRP, P], F32R, tag="xtg")
        for pi in range(GRP):
            gp = grp * GRP + pi
            xt = pIN.tile([P, NJ * P], F32R, tag="xt")
            nc.sync.dma_start(xt[:], xl[ds(gp * P, P), :])
            for jj in range(4):
                pt = psMX.tile([P, 512], F32, tag="mx")
                for i in range(4):
                    j = jj * 4 + i
                    nc.tensor.transpose(
                        pt[:, ds(i * P, P)].bitcast(F32R), xt[:, ds(j * P, P)], ident_r
                    )
                nc.vector.tensor_copy(
                    xtg[:, ts(jj, 4), pi, :], pt[:].rearrange("p (a b) -> p a b", a=4)
                )
        return xtg

    def phase_b(grp, xtg):
        """Projections for the group's 4 pairs; spill QT/KT, keep YV in SBUF."""
        for wname, wd, sp in (("q", wq, qsp), ("k", wk, ksp)):
            for j in range(NJ):
                wt = pW.tile([P, NJ, P], F32R, tag="wqk")
                nc.sync.dma_start(wt[:], wd[j])
                ps = psMX.tile([P, 512], F32, tag="mx")
                for kb in range(NJ):
                    nc.tensor.matmul(
                        ps[:], wt[:, kb], xtg[:, kb], start=(kb == 0), stop=(kb == NJ - 1)
                    )
                stg = pSTG.tile([P, GRP, P], F32R, tag="stg")
                nc.vector.tensor_copy(stg[:], ps[:].rearrange("p (g s) -> p g s", g=GRP))
                nc.sync.dma_start(sp[:, ds(grp * GRP, GRP), j, :], stg[:])
        for pi in range(GRP):
            yv_tiles[grp * GRP + pi] = pYV.tile(
                [P, NJ * P], F32, tag="yv", name=f"yv{grp * GRP + pi}"
            )
        for ec in range(8):
            wvt = pW.tile([P, NJ, 256], F32R, tag="wv")
            nc.sync.dma_start(wvt[:], wv[ec])
            for pi in range(GRP):
                gp = grp * GRP + pi
                ps = psMX.tile([P, 512], F32, tag="mx")
                for kb in range(NJ):
                    nc.tensor.matmul(
                        ps[:, :256], xtg[:, kb, pi], wvt[:, kb],
                        start=(kb == 0), stop=(kb == NJ - 1),
                    )
                nc.vector.tensor_copy(yv_tiles[gp][:, ds(ec * 256, 256)], ps[:, :256])

    def phase_c(gp):
        """Scores + softmax-over-q + attn + output for one pair."""
        qt = pQK.tile([P, NJ, P], F32R, tag="qt")
        nc.sync.dma_start(qt[:], qsp[:, gp])
        kt = pQK.tile([P, NJ, P], F32R, tag="kt")
        nc.sync.dma_start(kt[:], ksp[:, gp])
        yv = yv_tiles.pop(gp)
        acc = pACC.tile([P, NJ * P], F32, tag="acc")
        softs, vss = {}, {}
        for kj in range(NJ):
            soft = pSOFT.tile([P, 2048], BF16, tag="soft")
            pss, nms = [], []
            for h in range(2):
                ps = psSC.tile([P, 1024], F32, tag="sc")
                for c in range(2):
                    nc.tensor.matmul(
                        ps[:, ds(c * 512, 512)], kt[:, kj], qt[:, ts(h * 2 + c, 4)],
                        start=True, stop=True,
                    )
                nm = pST.tile([P, 1], F32, tag="nm")
                nc.vector.reduce_max(nm[:], ps[:], axis=AX, negate=True)
                pss.append(ps)
                nms.append(nm)
            ng = pST.tile([P, 1], F32, tag="ng")
            nc.vector.tensor_tensor(ng[:], nms[0][:], nms[1][:], mybir.AluOpType.min)
            ngs = pST.tile([P, 1], F32, tag="ngs")
            nc.vector.tensor_scalar_mul(ngs[:], ng[:], SCALE)
            lsum = pST.tile([P, 2], F32, tag="ls")
            for h in range(2):
                nc.scalar.activation(
                    soft[:, ds(h * 1024, 1024)], pss[h][:], EXP,
                    bias=ngs[:], scale=SCALE, accum_out=lsum[:, ds(h, 1)],
                )
            lt = pST.tile([P, 1], F32, tag="lt")
            nc.vector.reduce_sum(lt[:], lsum[:], axis=AX)
            rcp = pST.tile([P, 1], F32, tag="rcp")
            nc.vector.reciprocal(rcp[:], lt[:])
            vs = pVS.tile([P, P], BF16, tag="vs")
            nc.vector.tensor_scalar_mul(vs[:], yv[:, ts(kj, P)], rcp[:])
            softs[kj], vss[kj] = soft, vs
            if kj in GROUP_START:
                g0 = GROUP_START[kj]
                glen = kj - g0 + 1
                for h in range(2):
                    pa = psAT.tile([P, 1024], F32, tag="at")
                    for c in range(2):
                        for i in range(glen):
                            k2 = g0 + i
                            nc.tensor.matmul(
                                pa[:, ds(c * 512, 512)], vss[k2][:],
                                softs[k2][:, ds(h * 1024 + c * 512, 512)],
                                start=(i == 0), stop=(i == glen - 1),
                            )
                    if g0 == 0:
                        nc.vector.tensor_copy(acc[:, ds(h * 1024, 1024)], pa[:])
                    else:
                        nc.vector.tensor_add(
                            acc[:, ds(h * 1024, 1024)], acc[:, ds(h * 1024, 1024)], pa[:]
                        )
        for jj in range(4):
            pt = psMX.tile([P, 512], F32, tag="mx")
            for i in range(4):
                c = jj * 4 + i
                nc.tensor.transpose(pt[:, ds(i * P, P)], acc[:, ds(c * P, P)], ident[:])
            nc.scalar.copy(acc[:, ds(jj * 512, 512)], pt[:])
        nc.sync.dma_start(out[ds(gp * P, P), :], acc[:])

    for _rep in range(reps):
        for grp in range(NGRP):
            if "a" in parts:
                xtg = phase_a(grp)
            if "b" in parts:
                phase_b(grp, xtg)
            if "c" in parts:
                if "b" not in parts:
                    for pi in range(GRP):
                        t = pYV.tile(
                            [P, NJ * P], F32, tag="yv", name=f"yvx{grp * GRP + pi}"
                        )
                        nc.vector.memset(t[:], 0.5)
                        yv_tiles[grp * GRP + pi] = t
                for pi in range(GRP):
                    phase_c(grp * GRP + pi)


def build(reps=1, compile=True, parts="abc"):
    key = ("nc", reps, compile, parts)
    if key in _cache:
        return _cache[key]
    nc = bacc.Bacc("TRN2", target_bir_lowering=False, debug=False)
    xl = nc.dram_tensor("xl", [NPAIR * P, 2048], F32R, kind="ExternalInput").ap()
    wq = nc.dram_tensor("wq", [NJ, P, NJ, P], F32R, kind="ExternalInput").ap()
    wk = nc.dram_tensor("wk", [NJ, P, NJ, P], F32R, kind="ExternalInput").ap()
    wv = nc.dram_tensor("wv", [8, P, NJ, 256], F32R, kind="ExternalInput").ap()
    idr = nc.dram_tensor("idr", [P, P], F32R, kind="ExternalInput").ap()
    out = nc.dram_tensor("out", [NPAIR * P, 2048], F32, kind="ExternalOutput").ap()
    with tile.TileContext(nc) as tc:
        with ExitStack() as ctx:
            _emit(nc, tc, ctx, xl, wq, wk, wv, idr, out, reps=reps, parts=parts)
    if compile:
        nc.compile()
    _cache[key] = nc
    return nc


def kernel(x, w_query, w_key, w_value, _want_trace=False):
    x = np.ascontiguousarray(np.asarray(x, np.float32))
    wq = np.ascontiguousarray(np.asarray(w_query, np.float32))
    wk = np.ascontiguousarray(np.asarray(w_key, np.float32))
    wv = np.ascontiguousarray(np.asarray(w_value, np.float32))
    B, S, E = x.shape
    xf = x.reshape(B * S, E)
    nc = build()
    rows = NPAIR * P
    wq_t = np.ascontiguousarray(wq.reshape(NJ, P, NJ, P).transpose(2, 1, 0, 3))
    wk_t = np.ascontiguousarray(wk.reshape(NJ, P, NJ, P).transpose(2, 1, 0, 3))
    wv_t = np.ascontiguousarray(wv.reshape(NJ, P, 8, 256).transpose(2, 1, 0, 3))
    eye = np.eye(P, dtype=np.float32)
    in_maps = [
        dict(xl=np.ascontiguousarray(xf[c * rows:(c + 1) * rows]),
             wq=wq_t, wk=wk_t, wv=wv_t, idr=eye)
        for c in range(8)
    ]
    res = run_bass_kernel_spmd(nc, in_maps, core_ids=list(range(8)),
                               trace=_want_trace)
    outf = np.concatenate([r["out"] for r in res.results], axis=0)
    if _want_trace:
        kernel.last_result = res
    return outf.reshape(B, S, E)



# revision 12
# speedup vs baseline: 1.3246x; 1.3246x over previous
"""MultiHeadAttention Trainium2 Bass kernel, 8-core SPMD — v2.

Problem: B=4, S=2048, E=2048, H=16, Dh=128; reshape-based (not transposed)
head split:  q = (x@Wq).reshape(B,H,S,Dh) etc., softmax over the QUERY axis,
out = attn.reshape(B,S,E).

Sharding: flattening (B,S) rows, row-block gp (128 rows) of x@W is exactly
head pair gp=(b,h).  Core c owns 8 consecutive pairs -> contiguous x rows
[1024c, 1024c+1024) and the same output rows.  No collectives.

v2 changes vs v1:
  - Q/K projections kept in SBUF (pair-major [d, pair, j, s] layout), no
    DRAM spill round-trip.
  - Softmax uses per-1024-half biases (own max via one fused
    tensor_tensor_reduce with scale=-1), flash-style: the exp(m_h - m_glob)
    correction folds into the per-half V stationaries.  PSUM score tiles
    free right after their exp -> PE never waits on a global-max join.
  - 1/sqrt(Dh) pre-folded into w_query on the host.
  - Attention accumulated in PSUM over 4-kj segments (acc add in SBUF).
  - Engine balance: maxes/recips on DVE, exps on Act, factor/vs scaling on
    GPSIMD (SBUF-only), copies split DVE/Act.
  - PSUM: scores 3x[128,1024] (6 banks) + shared work pool 2x[128,512].
"""

import numpy as np
from contextlib import ExitStack

import concourse.bass as bass
import concourse.tile as tile
from concourse import bacc, mybir
from concourse.bass import ds, ts
from concourse.bass_utils import run_bass_kernel_spmd
from concourse.masks import make_identity

F32 = mybir.dt.float32
F32R = mybir.dt.float32r
BF16 = mybir.dt.bfloat16
AX = mybir.AxisListType.X
EXP = mybir.ActivationFunctionType.Exp
COPY = mybir.ActivationFunctionType.Copy
MAX = mybir.AluOpType.max
MIN = mybir.AluOpType.min
MULT = mybir.AluOpType.mult
ADD = mybir.AluOpType.add

P = 128
NPAIR = 8          # (b,h) pairs per core
GRP = 4            # pairs per group (weights streamed once per group)
NGRP = NPAIR // GRP
NJ = 16            # 128-col blocks in E
NSEG = 8           # kj per attention accumulation segment
SCALE = 1.0 / np.sqrt(128.0)
FBIG = 3.0e38

_cache = {}


def _emit(nc, tc, ctx, xl, wq, wk, wv, idr, out):
    sb = ctx.enter_context
    # SBUF pools
    pXIN = sb(tc.tile_pool(name="pxin", bufs=1))     # x row-block f32r   8K
    pXT = sb(tc.tile_pool(name="pxt", bufs=1))       # XT group           32K
    pWQK = sb(tc.tile_pool(name="pwqk", bufs=3))     # w half-tiles       4K*3
    pWV = sb(tc.tile_pool(name="pwv", bufs=2))       # wv half-tiles      8K*2
    pQT = sb(tc.tile_pool(name="pqt", bufs=1))       # QT group           32K
    pKT = sb(tc.tile_pool(name="pkt", bufs=1))       # KT group           32K
    pYV = sb(tc.tile_pool(name="pyv", bufs=4))       # yv bf16 per pair   4K*4
    pSOFT = sb(tc.tile_pool(name="psoft", bufs=17))  # soft halves bf16   2K*17
    pVS = sb(tc.tile_pool(name="pvs", bufs=18))      # vs bf16 per half   .25K*18
    pACC = sb(tc.tile_pool(name="pacc", bufs=2))     # attnT acc f32      8K*2
    pST = sb(tc.tile_pool(name="pst", bufs=6))       # small stats        tiny
    pSCR = sb(tc.tile_pool(name="pscr", bufs=1))     # TTR scratch        tiny
    pCONST = sb(tc.tile_pool(name="pconst", bufs=1))
    # PSUM pools: 3*2 + 2*1 = 8 banks
    psSC = sb(tc.tile_pool(name="pssc", bufs=3, space="PSUM"))   # [128,1024]
    psWK = sb(tc.tile_pool(name="pswk", bufs=2, space="PSUM"))   # [128,512]

    ident = pCONST.tile([P, P], F32, tag="ident")
    make_identity(nc, ident[:])
    identr = pCONST.tile([P, P], F32R, tag="identr")
    nc.sync.dma_start(identr[:], idr)
    ident_r = identr[:]

    scr2 = pSCR.tile([P, 2], F32, tag="scr2")

    yv_tiles = {}
    qt_tiles = {}
    kt_tiles = {}

    def phase_a(grp):
        """Transpose the group's x blocks into XT [128, kb, pair, s] f32r."""
        xtg = pXT.tile([P, NJ, GRP, P], F32R, tag="xtg")
        for pi in range(GRP):
            gp = grp * GRP + pi
            xt = pXIN.tile([P, NJ * P], F32R, tag="xt")
            nc.sync.dma_start(xt[:], xl[ds(gp * P, P), :])
            for jj in range(4):
                pt = psWK.tile([P, 512], F32, tag="wk")
                for i in range(4):
                    j = jj * 4 + i
                    nc.tensor.transpose(
                        pt[:, ds(i * P, P)].bitcast(F32R), xt[:, ds(j * P, P)], ident_r
                    )
                nc.vector.tensor_copy(
                    xtg[:, ts(jj, 4), pi, :], pt[:].rearrange("p (a b) -> p a b", a=4)
                )
        return xtg

    def phase_b_qk(grp, xtg):
        """Q/K projections, j-major over the group's 4 pairs.

        Output layout: qt/kt [128(d), pair, j, s] f32r kept in SBUF."""
        qtg = pQT.tile([P, GRP, NJ, P], F32R, tag="qtg")
        ktg = pKT.tile([P, GRP, NJ, P], F32R, tag="ktg")
        for j in range(NJ):
            for wd, dstg in ((wq, qtg), (wk, ktg)):
                ps = psWK.tile([P, 512], F32, tag="wk")
                for h in range(2):
                    wt = pWQK.tile([P, 8, P], F32R, tag="wqk")
                    nc.sync.dma_start(wt[:], wd[j, h])
                    for kb8 in range(8):
                        kb = h * 8 + kb8
                        nc.tensor.matmul(
                            ps[:], wt[:, kb8], xtg[:, kb],
                            start=(kb == 0), stop=(kb == NJ - 1),
                        )
                nc.vector.tensor_copy(
                    dstg[:, :, j, :], ps[:].rearrange("p (g s) -> p g s", g=GRP)
                )
        for pi in range(GRP):
            gp = grp * GRP + pi
            qt_tiles[gp] = qtg
            kt_tiles[gp] = ktg

    def phase_b_v(grp, xtg):
        """V projections: yv[pair] = [128(s), 2048(e)] bf16 in SBUF."""
        for pi in range(GRP):
            yv_tiles[grp * GRP + pi] = pYV.tile(
                [P, NJ * P], BF16, tag="yv", name=f"yv{grp * GRP + pi}"
            )
        for ec in range(8):
            wvts = []
            for h in range(2):
                wvt = pWV.tile([P, 8, 256], F32R, tag="wv")
                nc.sync.dma_start(wvt[:], wv[ec, h])
                wvts.append(wvt)
            for pi in range(GRP):
                gp = grp * GRP + pi
                ps = psWK.tile([P, 512], F32, tag="wk")
                for kb in range(NJ):
                    nc.tensor.matmul(
                        ps[:, :256], xtg[:, kb, pi], wvts[kb // 8][:, kb % 8],
                        start=(kb == 0), stop=(kb == NJ - 1),
                    )
                nc.scalar.copy(yv_tiles[gp][:, ds(ec * 256, 256)], ps[:, :256])

    def phase_c(gp, pi):
        """Scores + per-half-bias softmax-over-q + attn + output, one pair."""
        qtg = qt_tiles.pop(gp)
        ktg = kt_tiles.pop(gp)
        yv = yv_tiles.pop(gp)
        acc = pACC.tile([P, NJ * P], F32, tag="acc")
        for seg in range(NJ // NSEG):
            softs = {}
            vss = {}
            for i in range(NSEG):
                kj = seg * NSEG + i
                kt_st = ktg[:, pi, kj, :]
                nm2 = pST.tile([P, 2], F32, tag="nm2")
                ls2 = pST.tile([P, 2], F32, tag="ls2")
                f2 = pST.tile([P, 2], F32, tag="f2")
                for h in range(2):
                    ps = psSC.tile([P, 1024], F32, tag="sc")
                    for c in range(2):
                        nc.tensor.matmul(
                            ps[:, ds(c * 512, 512)], kt_st,
                            qtg[:, pi, ds(h * 8 + c * 4, 4), :],
                            start=True, stop=True,
                        )
                    # nm2[:,h] = -max over the 1024 cols of this half
                    nc.vector.reduce_max(
                        nm2[:, ds(h, 1)], ps[:], axis=AX, negate=True
                    )
                    soft = pSOFT.tile([P, 1024], BF16, tag="soft")
                    nc.scalar.activation(
                        soft[:], ps[:], EXP,
                        bias=nm2[:, ds(h, 1)], scale=1.0,
                        accum_out=ls2[:, ds(h, 1)],
                    )
                    softs[kj, h] = soft
                # nmmin = min(nm0, nm1) = -m_glob
                nmmin = pST.tile([P, 1], F32, tag="nmmin")
                nc.vector.tensor_tensor(nmmin[:], nm2[:, :1], nm2[:, 1:], op=MIN)
                dm2 = pST.tile([P, 2], F32, tag="dm2")
                nc.vector.tensor_scalar_sub(dm2[:], nm2[:], nmmin[:])
                # f_h = exp(m_h - m_glob) = exp(-dm2)
                nc.scalar.activation(f2[:], dm2[:], EXP, bias=0.0, scale=-1.0)
                # L = sum_h Ls[h] * f[h];  rcp = 1/L;  g_h = f_h * rcp
                lf = pST.tile([P, 2], F32, tag="lf")
                nc.vector.tensor_mul(lf[:], ls2[:], f2[:])
                lsum = pST.tile([P, 1], F32, tag="lsum")
                nc.vector.reduce_sum(lsum[:], lf[:], axis=AX)
                rcp = pST.tile([P, 1], F32, tag="rcp")
                nc.vector.reciprocal(rcp[:], lsum[:])
                g2 = pST.tile([P, 2], F32, tag="g2")
                nc.vector.tensor_scalar_mul(g2[:], f2[:], rcp[:])
                for h in range(2):
                    vs = pVS.tile([P, P], BF16, tag="vs")
                    nc.vector.tensor_scalar_mul(
                        vs[:], yv[:, ts(kj, P)], g2[:, ds(h, 1)]
                    )
                    vss[kj, h] = vs
            # attn for this segment: 4 q-quarters x NSEG kjs, accumulated in
            # PSUM then added into acc.
            for c in range(4):
                h = c // 2
                pa = psWK.tile([P, 512], F32, tag="wk")
                for i in range(NSEG):
                    kj = seg * NSEG + i
                    nc.tensor.matmul(
                        pa[:], vss[kj, h][:],
                        softs[kj, h][:, ds((c % 2) * 512, 512)],
                        start=(i == 0), stop=(i == NSEG - 1),
                    )
                if seg == 0:
                    nc.vector.tensor_copy(acc[:, ds(c * 512, 512)], pa[:])
                else:
                    nc.vector.tensor_add(
                        acc[:, ds(c * 512, 512)], acc[:, ds(c * 512, 512)], pa[:]
                    )
        # transpose acc (attnT) into output layout, in place per 512-chunk
        for jj in range(4):
            pt = psWK.tile([P, 512], F32, tag="wk")
            for i in range(4):
                cblk = jj * 4 + i
                nc.tensor.transpose(pt[:, ds(i * P, P)], acc[:, ds(cblk * P, P)], ident[:])
            nc.scalar.copy(acc[:, ds(jj * 512, 512)], pt[:])
        nc.sync.dma_start(out[ds(gp * P, P), :], acc[:])

    for grp in range(NGRP):
        xtg = phase_a(grp)
        phase_b_qk(grp, xtg)
        phase_b_v(grp, xtg)
        for pi in range(GRP):
            phase_c(grp * GRP + pi, pi)


def build(compile=True):
    key = ("nc", compile)
    if key in _cache:
        return _cache[key]
    nc = bacc.Bacc("TRN2", target_bir_lowering=False, debug=False)
    xl = nc.dram_tensor("xl", [NPAIR * P, 2048], F32R, kind="ExternalInput").ap()
    wq = nc.dram_tensor("wq", [NJ, 2, P, 8, P], F32R, kind="ExternalInput").ap()
    wk = nc.dram_tensor("wk", [NJ, 2, P, 8, P], F32R, kind="ExternalInput").ap()
    wv = nc.dram_tensor("wv", [8, 2, P, 8, 256], F32R, kind="ExternalInput").ap()
    idr = nc.dram_tensor("idr", [P, P], F32R, kind="ExternalInput").ap()
    out = nc.dram_tensor("out", [NPAIR * P, 2048], F32, kind="ExternalOutput").ap()
    with tile.TileContext(nc) as tc:
        with ExitStack() as ctx:
            _emit(nc, tc, ctx, xl, wq, wk, wv, idr, out)
    if compile:
        nc.compile()
    _cache[key] = nc
    return nc


def _prep_inputs(x, w_query, w_key, w_value):
    x = np.ascontiguousarray(np.asarray(x, np.float32))
    wq = np.asarray(w_query, np.float32)
    wk = np.asarray(w_key, np.float32)
    wv = np.asarray(w_value, np.float32)
    B, S, E = x.shape
    xf = x.reshape(B * S, E)
    # [j, half, p, kb8, q] ; 1/sqrt(Dh) folded into wq
    wq_t = np.ascontiguousarray(
        (wq * SCALE).reshape(NJ, P, NJ, P).transpose(2, 1, 0, 3)
        .reshape(NJ, P, 2, 8, P).transpose(0, 2, 1, 3, 4)
    )
    wk_t = np.ascontiguousarray(
        wk.reshape(NJ, P, NJ, P).transpose(2, 1, 0, 3)
        .reshape(NJ, P, 2, 8, P).transpose(0, 2, 1, 3, 4)
    )
    wv_t = np.ascontiguousarray(
        wv.reshape(NJ, P, 8, 256).transpose(2, 1, 0, 3)
        .reshape(8, P, 2, 8, 256).transpose(0, 2, 1, 3, 4)
    )
    eye = np.eye(P, dtype=np.float32)
    rows = NPAIR * P
    in_maps = [
        dict(xl=np.ascontiguousarray(xf[c * rows:(c + 1) * rows]),
             wq=wq_t, wk=wk_t, wv=wv_t, idr=eye)
        for c in range(8)
    ]
    return in_maps, (B, S, E)


def kernel(x, w_query, w_key, w_value, _want_trace=False):
    in_maps, (B, S, E) = _prep_inputs(x, w_query, w_key, w_value)
    nc = build()
    res = run_bass_kernel_spmd(nc, in_maps, core_ids=list(range(8)),
                               trace=_want_trace)
    outf = np.concatenate([r["out"] for r in res.results], axis=0)
    if _want_trace:
        kernel.last_result = res
    return outf.reshape(B, S, E)
